# revision 5
# baseline (speedup 1.0000x reference)
"""Trainium2 Bass kernel for nn_AttCo (dual-softmax co-attention block).

Contract: kernel(**inputs) takes the FULL unsharded inputs of reference.py's
setup_inputs() and returns the full output tuple (o1, o2).

Sharding: 8 cores = (batch b in 0..3) x (head-group g in 0..1, 4 heads each).
Each core computes its batch's Q/K/V projections for its 4 heads, both
softmax directions of attention, and a partial output projection over its
192 input channels. Host sums the two partials per batch and adds bias +
residual.

Device layout notes:
 - Everything stays channels-on-partitions [C, N] (N = 12^3 = 1728); the
   input tensor [C, 12, 12, 12] is natively this layout so there are no
   transposes anywhere.
 - Per head both S = Qh^T Kh (q on partitions) and S^T = Kh^T Qh (k on
   partitions) are computed on the PE; exp runs once per direction on the
   scalar engine with accum_out giving the softmax denominators for free.
 - The 1/denominator is folded into the tiny V^T tiles (per-partition
   scalar multiply) instead of normalizing the big score matrices.
 - All matmuls are bf16 (4x faster than fp32 on the PE); final output is
   fp32. The residual-dominated output makes bf16 rounding negligible.
"""

import math

import numpy as np
import ml_dtypes

import concourse.bass as bass
import concourse.bacc as bacc
import concourse.mybir as mybir
from concourse.tile import TileContext
from concourse.bass_utils import run_bass_kernel_spmd

BF16 = ml_dtypes.bfloat16

B, C, S = 4, 384, 12
N = S * S * S            # 1728
NH, HD = 8, C // 8       # 8 heads of dim 48
HPC = 4                  # heads per core
GW = HPC * HD            # 192 channels per core
SCALE = 1.0 / math.sqrt(HD)

# free-dim blocks of N (max 512 per matmul / PSUM bank)
BLK = [(0, 512), (512, 1024), (1024, 1536), (1536, 1728)]
# partition chunks of N (contraction / M tiling)
NCH = [(128 * i, min(128 * (i + 1), N)) for i in range((N + 127) // 128)]  # 14

_CACHE = {}


def _build_program():
    """Build the SPMD Bass program (identical on every core)."""
    fp32 = mybir.dt.float32
    bf16 = mybir.dt.bfloat16

    nc = bacc.Bacc(target_bir_lowering=False)

    x1 = nc.dram_tensor("x1", [C, N], bf16, kind="ExternalInput")
    x2 = nc.dram_tensor("x2", [C, N], bf16, kind="ExternalInput")
    wqT = nc.dram_tensor("wqT", [C, GW], bf16, kind="ExternalInput")
    wkT = nc.dram_tensor("wkT", [C, GW], bf16, kind="ExternalInput")
    wv1T = nc.dram_tensor("wv1T", [C, GW], bf16, kind="ExternalInput")
    wv2T = nc.dram_tensor("wv2T", [C, GW], bf16, kind="ExternalInput")
    wo1T = nc.dram_tensor("wo1T", [GW, C], bf16, kind="ExternalInput")
    wo2T = nc.dram_tensor("wo2T", [GW, C], bf16, kind="ExternalInput")
    out = nc.dram_tensor("out", [2, C, N], fp32, kind="ExternalOutput")

    with TileContext(nc) as tc:
        # ---- persistent SBUF tensors -------------------------------------
        with tc.tile_pool(name="wpool", bufs=1) as wpool, \
             tc.tile_pool(name="qkpool", bufs=1) as qkpool, \
             tc.tile_pool(name="vpool", bufs=1) as vpool, \
             tc.tile_pool(name="ctxpool", bufs=1) as ctxpool, \
             tc.tile_pool(name="redpool", bufs=1) as redpool:

            # weights in SBUF
            w_sb = {}
            for name, dram in [("wqT", wqT), ("wkT", wkT),
                               ("wv1T", wv1T), ("wv2T", wv2T)]:
                for c in range(3):
                    t = wpool.tile([128, GW], bf16, name=f"{name}_{c}")
                    nc.sync.dma_start(t[:, :], dram[128 * c:128 * (c + 1), :])
                    w_sb[(name, c)] = t
            wo_sb = {}
            for name, dram in [("wo1T", wo1T), ("wo2T", wo2T)]:
                for h in range(HPC):
                    t = wpool.tile([HD, C], bf16, name=f"{name}_{h}")
                    nc.sync.dma_start(t[:, :], dram[HD * h:HD * (h + 1), :])
                    wo_sb[(name, h)] = t

            # Q/K pair tiles: pair p holds head 2p at partitions 0:48 and
            # head 2p+1 at partitions 64:112 (padded so matmul base
            # partitions stay in {0, 64}).
            q_sb = [qkpool.tile([128, N], bf16, name=f"q_sb{p}") for p in range(2)]
            k_sb = [qkpool.tile([128, N], bf16, name=f"k_sb{p}") for p in range(2)]
            # V^T tiles: [n-partition chunk, head-major d columns]
            v1t_sb = vpool.tile([128, 192 * len(NCH)], bf16, name="v1t_sb")
            v2t_sb = vpool.tile([128, 192 * len(NCH)], bf16, name="v2t_sb")
            # per-head context outputs [48, N]
            ctx_sb = {(s, h): ctxpool.tile([HD, N], bf16, name=f"ctx{s}_{h}")
                      for s in range(2) for h in range(HPC)}
            # softmax denominators and reciprocals, one column per n-chunk
            nch = len(NCH)
            rsum = [redpool.tile([128, nch], fp32, name=f"rsum{h}") for h in range(HPC)]
            csum = [redpool.tile([128, nch], fp32, name=f"csum{h}") for h in range(HPC)]
            rrec = [redpool.tile([128, nch], fp32, name=f"rrec{h}") for h in range(HPC)]
            crec = [redpool.tile([128, nch], fp32, name=f"crec{h}") for h in range(HPC)]

            # ---- projections --------------------------------------------
            with tc.tile_pool(name="xpool", bufs=1) as xpool, \
                 tc.tile_pool(name="qkpsum", bufs=2, space="PSUM") as qkpsum, \
                 tc.tile_pool(name="vpsum", bufs=2, space="PSUM") as vpsum:

                x_sb = {}
                for name, dram in [("x1", x1), ("x2", x2)]:
                    for c in range(3):
                        t = xpool.tile([128, N], bf16, name=f"{name}_{c}")
                        nc.sync.dma_start(t[:, :], dram[128 * c:128 * (c + 1), :])
                        x_sb[(name, c)] = t

                # Q (from x1) and K (from x2): per pair, heads at col
                # offsets {0, 64} run concurrently on the PE.
                for wname, xname, dst in [("wqT", "x1", q_sb), ("wkT", "x2", k_sb)]:
                    for p in range(2):
                        for (j0, j1) in BLK:
                            ps = qkpsum.tile([128, 512], fp32, name="qk_ps")
                            for hh in range(2):
                                base = 64 * hh
                                col0 = GW // 2 * p + HD * hh
                                for c in range(3):
                                    nc.tensor.matmul(
                                        ps[base:base + HD, :j1 - j0],
                                        w_sb[(wname, c)][:, col0:col0 + HD],
                                        x_sb[(xname, c)][:, j0:j1],
                                        start=(c == 0), stop=(c == 2),
                                    )
                            nc.vector.tensor_copy(dst[p][:, j0:j1], ps[:, :j1 - j0])

                # V1^T (from x1) and V2^T (from x2): [n, d] layout
                for wname, xname, dst in [("wv1T", "x1", v1t_sb),
                                          ("wv2T", "x2", v2t_sb)]:
                    for i, (i0, i1) in enumerate(NCH):
                        cw = i1 - i0
                        ps = vpsum.tile([128, GW], fp32, name="v_ps")
                        for c in range(3):
                            nc.tensor.matmul(
                                ps[:cw, :],
                                x_sb[(xname, c)][:, i0:i1],
                                w_sb[(wname, c)][:, :],
                                start=(c == 0), stop=(c == 2),
                            )
                        nc.vector.tensor_copy(dst[:cw, GW * i:GW * (i + 1)], ps[:cw, :])

            # ---- attention ----------------------------------------------
            with tc.tile_pool(name="epool", bufs=30) as epool, \
                 tc.tile_pool(name="vspool", bufs=2 * len(NCH) + 4) as vspool, \
                 tc.tile_pool(name="spsum", bufs=1, space="PSUM") as spsum, \
                 tc.tile_pool(name="cpsum", bufs=2, space="PSUM") as cpsum:

                for h in range(HPC):
                    p, base = h // 2, 64 * (h % 2)
                    e_tiles, et_tiles, v1s, v2s = {}, {}, {}, {}

                    # S side: q on partitions -> e, rsum -> scaled V2^T
                    for i, (i0, i1) in enumerate(NCH):
                        cw = i1 - i0
                        sps = spsum.tile([128, N], fp32, name="sps")
                        for (j0, j1) in BLK:
                            nc.tensor.matmul(
                                sps[:cw, j0:j1],
                                q_sb[p][base:base + HD, i0:i1],
                                k_sb[p][base:base + HD, j0:j1],
                                start=True, stop=True,
                            )
                        e = epool.tile([128, N], bf16, name=f"e_{h}_{i}", tag="e")
                        nc.scalar.activation(
                            e[:cw, :], sps[:cw, :],
                            mybir.ActivationFunctionType.Exp,
                            accum_out=rsum[h][:cw, i:i + 1],
                        )
                        nc.vector.reciprocal(rrec[h][:cw, i:i + 1],
                                             rsum[h][:cw, i:i + 1])
                        vs = vspool.tile([128, HD], bf16, name=f"v2s_{h}_{i}", tag="vs")
                        nc.vector.tensor_scalar_mul(
                            vs[:cw, :],
                            v2t_sb[:cw, GW * i + HD * h:GW * i + HD * (h + 1)],
                            rrec[h][:cw, i:i + 1],
                        )
                        e_tiles[i], v2s[i] = e, vs

                    # S^T side: k on partitions -> eT, csum -> scaled V1^T
                    for i, (i0, i1) in enumerate(NCH):
                        cw = i1 - i0
                        sps = spsum.tile([128, N], fp32, name="sps")
                        for (j0, j1) in BLK:
                            nc.tensor.matmul(
                                sps[:cw, j0:j1],
                                k_sb[p][base:base + HD, i0:i1],
                                q_sb[p][base:base + HD, j0:j1],
                                start=True, stop=True,
                            )
                        et = epool.tile([128, N], bf16, name=f"et_{h}_{i}", tag="e")
                        nc.scalar.activation(
                            et[:cw, :], sps[:cw, :],
                            mybir.ActivationFunctionType.Exp,
                            accum_out=csum[h][:cw, i:i + 1],
                        )
                        nc.vector.reciprocal(crec[h][:cw, i:i + 1],
                                             csum[h][:cw, i:i + 1])
                        vs = vspool.tile([128, HD], bf16, name=f"v1s_{h}_{i}", tag="vs")
                        nc.vector.tensor_scalar_mul(
                            vs[:cw, :],
                            v1t_sb[:cw, GW * i + HD * h:GW * i + HD * (h + 1)],
                            crec[h][:cw, i:i + 1],
                        )
                        et_tiles[i], v1s[i] = et, vs

                    # ctx2[d, k] = sum_q V2s[q, d] e[q, k]
                    for (j0, j1) in BLK:
                        cps = cpsum.tile([HD, 512], fp32, name="c2ps")
                        for i, (i0, i1) in enumerate(NCH):
                            cw = i1 - i0
                            nc.tensor.matmul(
                                cps[:, :j1 - j0], v2s[i][:cw, :],
                                e_tiles[i][:cw, j0:j1],
                                start=(i == 0), stop=(i == len(NCH) - 1),
                            )
                        nc.vector.tensor_copy(ctx_sb[(1, h)][:, j0:j1],
                                              cps[:, :j1 - j0])

                    # ctx1[d, q] = sum_k V1s[k, d] eT[k, q]
                    for (j0, j1) in BLK:
                        cps = cpsum.tile([HD, 512], fp32, name="c1ps")
                        for i, (i0, i1) in enumerate(NCH):
                            cw = i1 - i0
                            nc.tensor.matmul(
                                cps[:, :j1 - j0], v1s[i][:cw, :],
                                et_tiles[i][:cw, j0:j1],
                                start=(i == 0), stop=(i == len(NCH) - 1),
                            )
                        nc.vector.tensor_copy(ctx_sb[(0, h)][:, j0:j1],
                                              cps[:, :j1 - j0])

            # ---- output projections (partial over this core's channels) --
            with tc.tile_pool(name="opsum", bufs=4, space="PSUM") as opsum, \
                 tc.tile_pool(name="ocopy", bufs=4) as ocopy:
                for s, wname in [(0, "wo1T"), (1, "wo2T")]:
                    for m in range(3):
                        for (j0, j1) in BLK:
                            ops = opsum.tile([128, 512], fp32, name="o_ps")
                            for h in range(HPC):
                                nc.tensor.matmul(
                                    ops[:, :j1 - j0],
                                    wo_sb[(wname, h)][:, 128 * m:128 * (m + 1)],
                                    ctx_sb[(s, h)][:, j0:j1],
                                    start=(h == 0), stop=(h == HPC - 1),
                                )
                            ob = ocopy.tile([128, 512], fp32, name="o_sb")
                            nc.vector.tensor_copy(ob[:, :j1 - j0], ops[:, :j1 - j0])
                            nc.sync.dma_start(
                                out[s, 128 * m:128 * (m + 1), j0:j1],
                                ob[:, :j1 - j0])

    nc.finalize()
    return nc


def kernel(x1, x2, Wq, bq, Wk, bk, Wv1, bv1, Wv2, bv2, Wo1, bo1, Wo2, bo2):
    x1 = np.asarray(x1, np.float32)
    x2 = np.asarray(x2, np.float32)
    assert not any(np.any(np.asarray(b)) for b in (bq, bk, bv1, bv2)), \
        "nonzero qkv biases not supported (spec guarantees zeros)"

    x1f = x1.reshape(B, C, N)
    x2f = x2.reshape(B, C, N)

    if "nc" not in _CACHE:
        _CACHE["nc"] = _build_program()
    nc = _CACHE["nc"]

    in_maps = []
    for core in range(8):
        b, g = core // 2, core % 2
        sl = slice(GW * g, GW * (g + 1))
        in_maps.append({
            "x1": x1f[b].astype(BF16),
            "x2": x2f[b].astype(BF16),
            "wqT": (np.asarray(Wq)[sl, :].T * SCALE).astype(BF16),
            "wkT": np.asarray(Wk)[sl, :].T.astype(BF16),
            "wv1T": np.asarray(Wv1)[sl, :].T.astype(BF16),
            "wv2T": np.asarray(Wv2)[sl, :].T.astype(BF16),
            "wo1T": np.asarray(Wo1)[:, sl].T.astype(BF16),
            "wo2T": np.asarray(Wo2)[:, sl].T.astype(BF16),
        })

    res = run_bass_kernel_spmd(nc, in_maps, list(range(8)))
    parts = [r["out"] for r in res.results]

    o1 = np.empty((B, C, N), np.float32)
    o2 = np.empty((B, C, N), np.float32)
    for b in range(B):
        o1[b] = parts[2 * b][0] + parts[2 * b + 1][0] + x1f[b]
        o2[b] = parts[2 * b][1] + parts[2 * b + 1][1] + x2f[b]
    o1 += np.asarray(bo1, np.float32)[None, :, None]
    o2 += np.asarray(bo2, np.float32)[None, :, None]
    return (o1.reshape(x1.shape), o2.reshape(x2.shape))


# revision 7
# speedup vs baseline: 1.3752x; 1.3752x over previous
"""Trainium2 Bass kernel for nn_AttCo (dual-softmax co-attention block).

Contract: kernel(**inputs) takes the FULL unsharded inputs of reference.py's
setup_inputs() and returns the full output tuple (o1, o2).

Sharding: 8 cores = (batch b in 0..3) x (head-group g in 0..1, 4 heads each).
Each core computes its batch's Q/K/V projections for its 4 heads, both
softmax directions of attention, and a partial output projection over its
192 input channels. Host sums the two partials per batch and adds bias +
residual.

Device layout notes (v2 — PE-packed):
 - Everything stays channels-on-partitions [C, N] (N = 12^3 = 1728); the
   input tensor [C, 12, 12, 12] is natively this layout, no transposes.
 - Q and K are written to BOTH partition halves {0:48, 64:112} so that
   even/odd q-chunks of the same head run CONCURRENTLY on disjoint PE
   row-groups; the scores PSUM is one [128, 3456] tile (7 banks) holding
   both chunks, with the odd half's k-blocks split at bank boundaries.
 - Per head both S = Qh^T Kh and S^T = Kh^T Qh are computed on the PE;
   exp runs once per direction on ScalarE with accum_out giving the
   softmax denominators for free. 1/denominator is folded into the tiny
   V^T tiles (per-partition scalar multiply).
 - Context matmuls pack even/odd contraction chunks into col-groups
   {0, 64} (concurrent); the even+odd merge is folded into the output
   projection via host-duplicated Wo rows (pad rows zero).
 - All matmuls bf16 (4x faster than fp32); output fp32. The residual-
   dominated output makes bf16 rounding negligible (~2e-5 rel).
"""

import math

import numpy as np
import ml_dtypes

import concourse.bass as bass
import concourse.bacc as bacc
import concourse.mybir as mybir
from concourse.tile import TileContext
from concourse.bass_utils import run_bass_kernel_spmd

BF16 = ml_dtypes.bfloat16

B, C, S = 4, 384, 12
N = S * S * S            # 1728
NH, HD = 8, C // 8       # 8 heads of dim 48
HPC = 4                  # heads per core
GW = HPC * HD            # 192 channels per core
SCALE = 1.0 / math.sqrt(HD)

# free-dim blocks of N for the "even" half (bank-aligned at tile col 0)
BLK = [(0, 512), (512, 1024), (1024, 1536), (1536, 1728)]
# blocks for the "odd" scores half living at col offset 1728 in the PSUM
# tile: each block must stay inside one 512-col PSUM bank, and 1728 is not
# bank-aligned, so split at the crossing (tile cols 2048 = bank 4).
OBLK = [(0, 320), (320, 832), (832, 1344), (1344, 1728)]
# partition chunks of N (contraction / M tiling)
NCH = [(128 * i, min(128 * (i + 1), N)) for i in range((N + 127) // 128)]  # 14

_CACHE = {}


def _build_program():
    fp32 = mybir.dt.float32
    bf16 = mybir.dt.bfloat16

    nc = bacc.Bacc(target_bir_lowering=False)

    x1 = nc.dram_tensor("x1", [C, N], bf16, kind="ExternalInput")
    x2 = nc.dram_tensor("x2", [C, N], bf16, kind="ExternalInput")
    wqT = nc.dram_tensor("wqT", [C, GW], bf16, kind="ExternalInput")
    wkT = nc.dram_tensor("wkT", [C, GW], bf16, kind="ExternalInput")
    wv1T = nc.dram_tensor("wv1T", [C, GW], bf16, kind="ExternalInput")
    wv2T = nc.dram_tensor("wv2T", [C, GW], bf16, kind="ExternalInput")
    # duplicated + zero-padded output weights: [head, 128, C] where rows
    # 0:48 and 64:112 both hold Wo^T[48h:48h+48, :]
    wo1T = nc.dram_tensor("wo1T", [HPC, 128, C], bf16, kind="ExternalInput")
    wo2T = nc.dram_tensor("wo2T", [HPC, 128, C], bf16, kind="ExternalInput")
    out = nc.dram_tensor("out", [2, C, N], fp32, kind="ExternalOutput")

    with TileContext(nc) as tc:
        with tc.tile_pool(name="wpool", bufs=1) as wpool, \
             tc.tile_pool(name="qkpool", bufs=1) as qkpool, \
             tc.tile_pool(name="vpool", bufs=1) as vpool, \
             tc.tile_pool(name="ctxpool", bufs=1) as ctxpool, \
             tc.tile_pool(name="redpool", bufs=1) as redpool:

            w_sb = {}
            for name, dram in [("wqT", wqT), ("wkT", wkT),
                               ("wv1T", wv1T), ("wv2T", wv2T)]:
                for c in range(3):
                    t = wpool.tile([128, GW], bf16, name=f"{name}_{c}")
                    nc.sync.dma_start(t[:, :], dram[128 * c:128 * (c + 1), :])
                    w_sb[(name, c)] = t
            wo_sb = {}
            for name, dram in [("wo1T", wo1T), ("wo2T", wo2T)]:
                for h in range(HPC):
                    t = wpool.tile([128, C], bf16, name=f"{name}_{h}")
                    nc.sync.dma_start(t[:, :], dram[h])
                    wo_sb[(name, h)] = t

            # Q/K per-head tiles duplicated into both partition halves
            q_sb = [qkpool.tile([128, N], bf16, name=f"q_sb{h}") for h in range(HPC)]
            k_sb = [qkpool.tile([128, N], bf16, name=f"k_sb{h}") for h in range(HPC)]
            v1t_sb = vpool.tile([128, 192 * len(NCH)], bf16, name="v1t_sb")
            v2t_sb = vpool.tile([128, 192 * len(NCH)], bf16, name="v2t_sb")
            # per-head context tiles: even half at rows 0:48, odd at 64:112;
            # pad rows (48:64, 112:128) must stay zero -> memset once.
            ctx_sb = {}
            for s in range(2):
                for h in range(HPC):
                    t = ctxpool.tile([128, N], bf16, name=f"ctx{s}_{h}")
                    # zero the pad rows (48:64, 112:128); DVE partition bases
                    # must be 32-aligned, so clear [32:64]/[96:128] and let
                    # the later context copies overwrite the live parts.
                    nc.vector.memset(t[32:64, :], 0.0)
                    nc.vector.memset(t[96:128, :], 0.0)
                    ctx_sb[(s, h)] = t
            nch = len(NCH)
            rsum = [redpool.tile([128, nch], fp32, name=f"rsum{h}") for h in range(HPC)]
            csum = [redpool.tile([128, nch], fp32, name=f"csum{h}") for h in range(HPC)]
            rrec = [redpool.tile([128, nch], fp32, name=f"rrec{h}") for h in range(HPC)]
            crec = [redpool.tile([128, nch], fp32, name=f"crec{h}") for h in range(HPC)]

            # ---- projections --------------------------------------------
            with tc.tile_pool(name="xpool", bufs=1) as xpool, \
                 tc.tile_pool(name="qkpsum", bufs=2, space="PSUM") as qkpsum, \
                 tc.tile_pool(name="vpsum", bufs=2, space="PSUM") as vpsum:

                x_sb = {}
                for name, dram in [("x1", x1), ("x2", x2)]:
                    for c in range(3):
                        t = xpool.tile([128, N], bf16, name=f"{name}_{c}")
                        nc.sync.dma_start(t[:, :], dram[128 * c:128 * (c + 1), :])
                        x_sb[(name, c)] = t

                # Q (x1) / K (x2): per head, write to BOTH halves {0, 64}
                for wname, xname, dst in [("wqT", "x1", q_sb), ("wkT", "x2", k_sb)]:
                    for h in range(HPC):
                        col0 = HD * h
                        for (j0, j1) in BLK:
                            ps = qkpsum.tile([128, 512], fp32, name="qk_ps")
                            for base in (0, 64):
                                for c in range(3):
                                    nc.tensor.matmul(
                                        ps[base:base + HD, :j1 - j0],
                                        w_sb[(wname, c)][:, col0:col0 + HD],
                                        x_sb[(xname, c)][:, j0:j1],
                                        start=(c == 0), stop=(c == 2),
                                    )
                            nc.vector.tensor_copy(dst[h][:, j0:j1], ps[:, :j1 - j0])

                for wname, xname, dst in [("wv1T", "x1", v1t_sb),
                                          ("wv2T", "x2", v2t_sb)]:
                    for i, (i0, i1) in enumerate(NCH):
                        cw = i1 - i0
                        ps = vpsum.tile([128, GW], fp32, name="v_ps")
                        for c in range(3):
                            nc.tensor.matmul(
                                ps[:cw, :],
                                x_sb[(xname, c)][:, i0:i1],
                                w_sb[(wname, c)][:, :],
                                start=(c == 0), stop=(c == 2),
                            )
                        nc.vector.tensor_copy(dst[:cw, GW * i:GW * (i + 1)], ps[:cw, :])

            # ---- attention ----------------------------------------------
            with tc.tile_pool(name="epool", bufs=32) as epool, \
                 tc.tile_pool(name="vspool", bufs=32) as vspool, \
                 tc.tile_pool(name="spsum", bufs=1, space="PSUM") as spsum, \
                 tc.tile_pool(name="cpsum", bufs=1, space="PSUM") as cpsum:

                def scores_side(h, lhs, rhs, e_prefix, acc, rec, vt, vs_out):
                    """One direction: S-chunks (even/odd packed), exp+accum,
                    reciprocal, scaled-V tiles. lhs/rhs are the duplicated
                    [128, N] tiles (contraction operand slices at base 0/64).
                    """
                    e_tiles = {}
                    for ic in range(0, nch, 2):
                        sps = spsum.tile([128, 2 * N], fp32, name="sps")
                        for half, i in ((0, ic), (1, ic + 1)):
                            if i >= nch:
                                continue
                            i0, i1 = NCH[i]
                            cw = i1 - i0
                            base = 64 * half
                            off = N * half
                            blocks = BLK if half == 0 else OBLK
                            for (j0, j1) in blocks:
                                nc.tensor.matmul(
                                    sps[:cw, off + j0:off + j1],
                                    lhs[base:base + HD, i0:i1],
                                    rhs[base:base + HD, j0:j1],
                                    start=True, stop=True,
                                )
                        for half, i in ((0, ic), (1, ic + 1)):
                            if i >= nch:
                                continue
                            i0, i1 = NCH[i]
                            cw = i1 - i0
                            off = N * half
                            e = epool.tile([128, N], bf16,
                                           name=f"{e_prefix}_{h}_{i}", tag="e")
                            nc.scalar.activation(
                                e[:cw, :], sps[:cw, off:off + N],
                                mybir.ActivationFunctionType.Exp,
                                accum_out=acc[:cw, i:i + 1],
                            )
                            nc.vector.reciprocal(rec[:cw, i:i + 1], acc[:cw, i:i + 1])
                            vs = vspool.tile([128, HD], bf16,
                                             name=f"vs_{e_prefix}_{h}_{i}", tag="vs")
                            nc.vector.tensor_scalar_mul(
                                vs[:cw, :],
                                vt[:cw, GW * i + HD * h:GW * i + HD * (h + 1)],
                                rec[:cw, i:i + 1],
                            )
                            e_tiles[i] = e
                            vs_out[i] = vs
                    return e_tiles

                def ctx_accum(h, s, vs, e_tiles):
                    """ctx[d, n] = sum_i vs_i^T e_i with even/odd chunks
                    packed into col-groups {0, 64}; merge happens in the
                    output projection via duplicated Wo rows."""
                    for (j0, j1) in BLK:
                        cps = cpsum.tile([128, 512], fp32, name="cps")
                        for i, (i0, i1) in enumerate(NCH):
                            cw = i1 - i0
                            base = 64 * (i % 2)
                            nc.tensor.matmul(
                                cps[base:base + HD, :j1 - j0],
                                vs[i][:cw, :],
                                e_tiles[i][:cw, j0:j1],
                                start=(i < 2), stop=(i >= nch - 2),
                            )
                        dst = ctx_sb[(s, h)]
                        nc.vector.tensor_copy(dst[0:48, j0:j1], cps[0:48, :j1 - j0])
                        nc.vector.tensor_copy(dst[64:112, j0:j1], cps[64:112, :j1 - j0])

                for h in range(HPC):
                    v1s, v2s = {}, {}
                    e_t = scores_side(h, q_sb[h], k_sb[h], "e", rsum[h],
                                      rrec[h], v2t_sb, v2s)
                    et_t = scores_side(h, k_sb[h], q_sb[h], "et", csum[h],
                                       crec[h], v1t_sb, v1s)
                    ctx_accum(h, 1, v2s, e_t)     # ctx2 consumes e
                    ctx_accum(h, 0, v1s, et_t)    # ctx1 consumes eT

            # ---- output projections -------------------------------------
            with tc.tile_pool(name="opsum", bufs=4, space="PSUM") as opsum, \
                 tc.tile_pool(name="ocopy", bufs=4) as ocopy:
                for s, wname in [(0, "wo1T"), (1, "wo2T")]:
                    for m in range(3):
                        for (j0, j1) in BLK:
                            ops = opsum.tile([128, 512], fp32, name="o_ps")
                            for h in range(HPC):
                                nc.tensor.matmul(
                                    ops[:, :j1 - j0],
                                    wo_sb[(wname, h)][:, 128 * m:128 * (m + 1)],
                                    ctx_sb[(s, h)][:, j0:j1],
                                    start=(h == 0), stop=(h == HPC - 1),
                                )
                            ob = ocopy.tile([128, 512], fp32, name="o_sb")
                            nc.vector.tensor_copy(ob[:, :j1 - j0], ops[:, :j1 - j0])
                            nc.sync.dma_start(
                                out[s, 128 * m:128 * (m + 1), j0:j1],
                                ob[:, :j1 - j0])

    nc.finalize()
    return nc


def kernel(x1, x2, Wq, bq, Wk, bk, Wv1, bv1, Wv2, bv2, Wo1, bo1, Wo2, bo2):
    x1 = np.asarray(x1, np.float32)
    x2 = np.asarray(x2, np.float32)
    assert not any(np.any(np.asarray(b)) for b in (bq, bk, bv1, bv2)), \
        "nonzero qkv biases not supported (spec guarantees zeros)"

    x1f = x1.reshape(B, C, N)
    x2f = x2.reshape(B, C, N)

    if "nc" not in _CACHE:
        _CACHE["nc"] = _build_program()
    nc = _CACHE["nc"]

    in_maps = [core_inputs(core, x1f, x2f, Wq, Wk, Wv1, Wv2, Wo1, Wo2)
               for core in range(8)]

    res = run_bass_kernel_spmd(nc, in_maps, list(range(8)))
    parts = [r["out"] for r in res.results]

    o1 = np.empty((B, C, N), np.float32)
    o2 = np.empty((B, C, N), np.float32)
    for b in range(B):
        o1[b] = parts[2 * b][0] + parts[2 * b + 1][0] + x1f[b]
        o2[b] = parts[2 * b][1] + parts[2 * b + 1][1] + x2f[b]
    o1 += np.asarray(bo1, np.float32)[None, :, None]
    o2 += np.asarray(bo2, np.float32)[None, :, None]
    return (o1.reshape(x1.shape), o2.reshape(x2.shape))


def core_inputs(core, x1f, x2f, Wq, Wk, Wv1, Wv2, Wo1, Wo2):
    b, g = core // 2, core % 2
    sl = slice(GW * g, GW * (g + 1))

    def wo_dup(Wo):
        woT = np.asarray(Wo)[:, sl].T.astype(np.float32)  # [192, 384]
        t = np.zeros((HPC, 128, C), np.float32)
        for h in range(HPC):
            blk = woT[HD * h:HD * (h + 1), :]
            t[h, 0:48] = blk
            t[h, 64:112] = blk
        return t.astype(BF16)

    return {
        "x1": x1f[b].astype(BF16),
        "x2": x2f[b].astype(BF16),
        "wqT": (np.asarray(Wq)[sl, :].T * SCALE).astype(BF16),
        "wkT": np.asarray(Wk)[sl, :].T.astype(BF16),
        "wv1T": np.asarray(Wv1)[sl, :].T.astype(BF16),
        "wv2T": np.asarray(Wv2)[sl, :].T.astype(BF16),
        "wo1T": wo_dup(Wo1),
        "wo2T": wo_dup(Wo2),
    }


# revision 12
# speedup vs baseline: 1.4126x; 1.0272x over previous
"""Trainium2 Bass kernel for nn_AttCo (dual-softmax co-attention block).

Contract: kernel(**inputs) takes the FULL unsharded inputs of reference.py's
setup_inputs() and returns the full output tuple (o1, o2).

Sharding: 8 cores = (batch b in 0..3) x (head-group g in 0..1, 4 heads each).
Each core computes its batch's Q/K/V projections for its 4 heads, both
softmax directions of attention, and a partial output projection over its
192 input channels. Host sums the two partials per batch and adds bias +
residual.

Device layout notes (v2 — PE-packed):
 - Everything stays channels-on-partitions [C, N] (N = 12^3 = 1728); the
   input tensor [C, 12, 12, 12] is natively this layout, no transposes.
 - Q and K are written to BOTH partition halves {0:48, 64:112} so that
   even/odd q-chunks of the same head run CONCURRENTLY on disjoint PE
   row-groups; the scores PSUM is one [128, 3456] tile (7 banks) holding
   both chunks, with the odd half's k-blocks split at bank boundaries.
 - Per head both S = Qh^T Kh and S^T = Kh^T Qh are computed on the PE;
   exp runs once per direction on ScalarE with accum_out giving the
   softmax denominators for free. 1/denominator is folded into the tiny
   V^T tiles (per-partition scalar multiply).
 - Context matmuls pack even/odd contraction chunks into col-groups
   {0, 64} (concurrent); the even+odd merge is folded into the output
   projection via host-duplicated Wo rows (pad rows zero).
 - All matmuls bf16 (4x faster than fp32); output fp32. The residual-
   dominated output makes bf16 rounding negligible (~2e-5 rel).
"""

import math

import numpy as np
import ml_dtypes

import concourse.bass as bass
import concourse.bacc as bacc
import concourse.mybir as mybir
from concourse.tile import TileContext
from concourse.bass_utils import run_bass_kernel_spmd

BF16 = ml_dtypes.bfloat16

B, C, S = 4, 384, 12
N = S * S * S            # 1728
NH, HD = 8, C // 8       # 8 heads of dim 48
HPC = 4                  # heads per core
GW = HPC * HD            # 192 channels per core
SCALE = 1.0 / math.sqrt(HD)

# free-dim blocks of N for the "even" half (bank-aligned at tile col 0)
BLK = [(0, 512), (512, 1024), (1024, 1536), (1536, 1728)]
# blocks for the "odd" scores half living at col offset 1728 in the PSUM
# tile: each block must stay inside one 512-col PSUM bank, and 1728 is not
# bank-aligned, so split at the crossing (tile cols 2048 = bank 4).
OBLK = [(0, 320), (320, 832), (832, 1344), (1344, 1728)]
# partition chunks of N (contraction / M tiling)
NCH = [(128 * i, min(128 * (i + 1), N)) for i in range((N + 127) // 128)]  # 14

_CACHE = {}


def _build_program():
    fp32 = mybir.dt.float32
    bf16 = mybir.dt.bfloat16

    nc = bacc.Bacc(target_bir_lowering=False)

    x1 = nc.dram_tensor("x1", [C, N], bf16, kind="ExternalInput")
    x2 = nc.dram_tensor("x2", [C, N], bf16, kind="ExternalInput")
    wqT = nc.dram_tensor("wqT", [C, GW], bf16, kind="ExternalInput")
    wkT = nc.dram_tensor("wkT", [C, GW], bf16, kind="ExternalInput")
    wv1T = nc.dram_tensor("wv1T", [C, GW], bf16, kind="ExternalInput")
    wv2T = nc.dram_tensor("wv2T", [C, GW], bf16, kind="ExternalInput")
    # duplicated + zero-padded output weights: [head, 128, C] where rows
    # 0:48 and 64:112 both hold Wo^T[48h:48h+48, :]
    wo1T = nc.dram_tensor("wo1T", [HPC, 128, C], bf16, kind="ExternalInput")
    wo2T = nc.dram_tensor("wo2T", [HPC, 128, C], bf16, kind="ExternalInput")
    out = nc.dram_tensor("out", [2, C, N], fp32, kind="ExternalOutput")

    with TileContext(nc) as tc:
        with tc.tile_pool(name="wpool", bufs=1) as wpool, \
             tc.tile_pool(name="qkpool", bufs=1) as qkpool, \
             tc.tile_pool(name="vpool", bufs=1) as vpool, \
             tc.tile_pool(name="ctxpool", bufs=1) as ctxpool, \
             tc.tile_pool(name="redpool", bufs=1) as redpool:

            w_sb = {}
            for name, dram in [("wqT", wqT), ("wkT", wkT),
                               ("wv1T", wv1T), ("wv2T", wv2T)]:
                for c in range(3):
                    t = wpool.tile([128, GW], bf16, name=f"{name}_{c}")
                    nc.sync.dma_start(t[:, :], dram[128 * c:128 * (c + 1), :])
                    w_sb[(name, c)] = t
            wo_sb = {}
            for name, dram in [("wo1T", wo1T), ("wo2T", wo2T)]:
                for h in range(HPC):
                    t = wpool.tile([128, C], bf16, name=f"{name}_{h}")
                    nc.sync.dma_start(t[:, :], dram[h])
                    wo_sb[(name, h)] = t

            # Q/K per-head tiles duplicated into both partition halves
            q_sb = [qkpool.tile([128, N], bf16, name=f"q_sb{h}") for h in range(HPC)]
            k_sb = [qkpool.tile([128, N], bf16, name=f"k_sb{h}") for h in range(HPC)]
            v1t_sb = vpool.tile([128, 192 * len(NCH)], bf16, name="v1t_sb")
            v2t_sb = vpool.tile([128, 192 * len(NCH)], bf16, name="v2t_sb")
            # per-head context tiles: even half at rows 0:48, odd at 64:112;
            # pad rows (48:64, 112:128) must stay zero -> memset once.
            ctx_sb = {}
            for s in range(2):
                for h in range(HPC):
                    t = ctxpool.tile([128, N], bf16, name=f"ctx{s}_{h}")
                    # zero the pad rows (48:64, 112:128); DVE partition bases
                    # must be 32-aligned, so clear [32:64]/[96:128] and let
                    # the later context copies overwrite the live parts.
                    nc.vector.memset(t[32:64, :], 0.0)
                    nc.vector.memset(t[96:128, :], 0.0)
                    ctx_sb[(s, h)] = t
            nch = len(NCH)
            rsum = [redpool.tile([128, nch], fp32, name=f"rsum{h}") for h in range(HPC)]
            csum = [redpool.tile([128, nch], fp32, name=f"csum{h}") for h in range(HPC)]
            rrec = [redpool.tile([128, nch], fp32, name=f"rrec{h}") for h in range(HPC)]
            crec = [redpool.tile([128, nch], fp32, name=f"crec{h}") for h in range(HPC)]

            # ---- projections --------------------------------------------
            with tc.tile_pool(name="xpool", bufs=1) as xpool, \
                 tc.tile_pool(name="qkpsum", bufs=2, space="PSUM") as qkpsum, \
                 tc.tile_pool(name="vpsum", bufs=2, space="PSUM") as vpsum:

                x_sb = {}
                for name, dram in [("x1", x1), ("x2", x2)]:
                    for c in range(3):
                        t = xpool.tile([128, N], bf16, name=f"{name}_{c}")
                        nc.sync.dma_start(t[:, :], dram[128 * c:128 * (c + 1), :])
                        x_sb[(name, c)] = t

                # Q (x1) / K (x2): per head, write to BOTH halves {0, 64}.
                # Loop heads outermost so head 0's S-matmuls (and the exp
                # pipeline) can start while later heads still project.
                for h in range(HPC):
                    for wname, xname, dst in [("wqT", "x1", q_sb),
                                              ("wkT", "x2", k_sb)]:
                        col0 = HD * h
                        for (j0, j1) in BLK:
                            ps = qkpsum.tile([128, 512], fp32, name="qk_ps")
                            for base in (0, 64):
                                for c in range(3):
                                    nc.tensor.matmul(
                                        ps[base:base + HD, :j1 - j0],
                                        w_sb[(wname, c)][:, col0:col0 + HD],
                                        x_sb[(xname, c)][:, j0:j1],
                                        start=(c == 0), stop=(c == 2),
                                    )
                            nc.vector.tensor_copy(dst[h][:, j0:j1], ps[:, :j1 - j0])

                for wname, xname, dst in [("wv1T", "x1", v1t_sb),
                                          ("wv2T", "x2", v2t_sb)]:
                    for i, (i0, i1) in enumerate(NCH):
                        cw = i1 - i0
                        ps = vpsum.tile([128, GW], fp32, name="v_ps")
                        for c in range(3):
                            nc.tensor.matmul(
                                ps[:cw, :],
                                x_sb[(xname, c)][:, i0:i1],
                                w_sb[(wname, c)][:, :],
                                start=(c == 0), stop=(c == 2),
                            )
                        nc.vector.tensor_copy(dst[:cw, GW * i:GW * (i + 1)], ps[:cw, :])

            # ---- attention ----------------------------------------------
            with tc.tile_pool(name="epool", bufs=32) as epool, \
                 tc.tile_pool(name="vspool", bufs=32) as vspool, \
                 tc.tile_pool(name="spsum", bufs=1, space="PSUM") as spsum, \
                 tc.tile_pool(name="cpsum", bufs=1, space="PSUM") as cpsum:

                def scores_side(h, lhs, rhs, e_prefix, acc, rec, vt, vs_out,
                                e_tiles):
                    """One direction: S-chunks (even/odd packed), exp+accum,
                    reciprocal, scaled-V tiles. lhs/rhs are the duplicated
                    [128, N] tiles (contraction operand slices at base 0/64).
                    Generator: yields after each chunk-pair so the driver can
                    interleave emission with context matmuls (keeps both the
                    PE dense and the scalar engine fed).
                    """
                    for ic in range(0, nch, 2):
                        sps = spsum.tile([128, 2 * N], fp32, name="sps")
                        for half, i in ((0, ic), (1, ic + 1)):
                            if i >= nch:
                                continue
                            i0, i1 = NCH[i]
                            cw = i1 - i0
                            base = 64 * half
                            off = N * half
                            blocks = BLK if half == 0 else OBLK
                            for (j0, j1) in blocks:
                                nc.tensor.matmul(
                                    sps[:cw, off + j0:off + j1],
                                    lhs[base:base + HD, i0:i1],
                                    rhs[base:base + HD, j0:j1],
                                    start=True, stop=True,
                                )
                        for half, i in ((0, ic), (1, ic + 1)):
                            if i >= nch:
                                continue
                            i0, i1 = NCH[i]
                            cw = i1 - i0
                            off = N * half
                            e = epool.tile([128, N], bf16,
                                           name=f"{e_prefix}_{h}_{i}", tag="e")
                            nc.scalar.activation(
                                e[:cw, :], sps[:cw, off:off + N],
                                mybir.ActivationFunctionType.Exp,
                                accum_out=acc[:cw, i:i + 1],
                            )
                            nc.vector.reciprocal(rec[:cw, i:i + 1], acc[:cw, i:i + 1])
                            vs = vspool.tile([128, HD], bf16,
                                             name=f"vs_{e_prefix}_{h}_{i}", tag="vs")
                            nc.vector.tensor_scalar_mul(
                                vs[:cw, :],
                                vt[:cw, GW * i + HD * h:GW * i + HD * (h + 1)],
                                rec[:cw, i:i + 1],
                            )
                            e_tiles[i] = e
                            vs_out[i] = vs
                        yield

                def ctx_accum(h, s, vs, e_tiles):
                    """ctx[d, n] = sum_i vs_i^T e_i with even/odd chunks
                    packed into col-groups {0, 64}; merge happens in the
                    output projection via duplicated Wo rows. Generator:
                    yields after each output block."""
                    for (j0, j1) in BLK:
                        cps = cpsum.tile([128, 512], fp32, name="cps")
                        for i, (i0, i1) in enumerate(NCH):
                            cw = i1 - i0
                            base = 64 * (i % 2)
                            nc.tensor.matmul(
                                cps[base:base + HD, :j1 - j0],
                                vs[i][:cw, :],
                                e_tiles[i][:cw, j0:j1],
                                start=(i < 2), stop=(i >= nch - 2),
                            )
                        dst = ctx_sb[(s, h)]
                        nc.vector.tensor_copy(dst[0:48, j0:j1], cps[0:48, :j1 - j0])
                        nc.vector.tensor_copy(dst[64:112, j0:j1], cps[64:112, :j1 - j0])
                        yield

                def drive(primary, secondary):
                    """Alternate emission: one primary group, one secondary
                    group, then exhaust both."""
                    for _ in primary:
                        if secondary is not None:
                            next(secondary, None)
                    if secondary is not None:
                        for _ in secondary:
                            pass

                # Software pipeline across heads:
                #   S_h -> [ST_h | ctx2_h] -> [S_{h+1} | ctx1_h] -> ...
                e_t, et_t, v1s, v2s = {}, {}, {}, {}
                s_gen = scores_side(0, q_sb[0], k_sb[0], "e", rsum[0],
                                    rrec[0], v2t_sb, v2s, e_t)
                pending_ctx1 = None
                drive(s_gen, None)
                for h in range(HPC):
                    st_gen = scores_side(h, k_sb[h], q_sb[h], "et", csum[h],
                                         crec[h], v1t_sb, v1s, et_t)
                    drive(st_gen, ctx_accum(h, 1, v2s, e_t))  # ST | ctx2
                    ctx1 = ctx_accum(h, 0, v1s, et_t)
                    if h + 1 < HPC:
                        e_t, et_t, v1s, v2s = {}, {}, {}, {}
                        nxt = scores_side(h + 1, q_sb[h + 1], k_sb[h + 1],
                                          "e", rsum[h + 1], rrec[h + 1],
                                          v2t_sb, v2s, e_t)
                        drive(nxt, ctx1)                      # S' | ctx1
                    else:
                        drive(ctx1, None)

            # ---- output projections -------------------------------------
            with tc.tile_pool(name="opsum", bufs=4, space="PSUM") as opsum, \
                 tc.tile_pool(name="ocopy", bufs=4) as ocopy:
                # ctx2 tiles (s=1) are complete before the last ctx1, so
                # emit the o2 projection first to overlap the pipeline tail.
                for s, wname in [(1, "wo2T"), (0, "wo1T")]:
                    for m in range(3):
                        for (j0, j1) in BLK:
                            ops = opsum.tile([128, 512], fp32, name="o_ps")
                            for h in range(HPC):
                                nc.tensor.matmul(
                                    ops[:, :j1 - j0],
                                    wo_sb[(wname, h)][:, 128 * m:128 * (m + 1)],
                                    ctx_sb[(s, h)][:, j0:j1],
                                    start=(h == 0), stop=(h == HPC - 1),
                                )
                            ob = ocopy.tile([128, 512], fp32, name="o_sb")
                            nc.vector.tensor_copy(ob[:, :j1 - j0], ops[:, :j1 - j0])
                            nc.sync.dma_start(
                                out[s, 128 * m:128 * (m + 1), j0:j1],
                                ob[:, :j1 - j0])

    nc.finalize()
    return nc


def kernel(x1, x2, Wq, bq, Wk, bk, Wv1, bv1, Wv2, bv2, Wo1, bo1, Wo2, bo2):
    x1 = np.asarray(x1, np.float32)
    x2 = np.asarray(x2, np.float32)
    assert not any(np.any(np.asarray(b)) for b in (bq, bk, bv1, bv2)), \
        "nonzero qkv biases not supported (spec guarantees zeros)"

    x1f = x1.reshape(B, C, N)
    x2f = x2.reshape(B, C, N)

    if "nc" not in _CACHE:
        _CACHE["nc"] = _build_program()
    nc = _CACHE["nc"]

    in_maps = [core_inputs(core, x1f, x2f, Wq, Wk, Wv1, Wv2, Wo1, Wo2)
               for core in range(8)]

    res = run_bass_kernel_spmd(nc, in_maps, list(range(8)))
    parts = [r["out"] for r in res.results]

    o1 = np.empty((B, C, N), np.float32)
    o2 = np.empty((B, C, N), np.float32)
    for b in range(B):
        o1[b] = parts[2 * b][0] + parts[2 * b + 1][0] + x1f[b]
        o2[b] = parts[2 * b][1] + parts[2 * b + 1][1] + x2f[b]
    o1 += np.asarray(bo1, np.float32)[None, :, None]
    o2 += np.asarray(bo2, np.float32)[None, :, None]
    return (o1.reshape(x1.shape), o2.reshape(x2.shape))


def core_inputs(core, x1f, x2f, Wq, Wk, Wv1, Wv2, Wo1, Wo2):
    b, g = core // 2, core % 2
    sl = slice(GW * g, GW * (g + 1))

    def wo_dup(Wo):
        woT = np.asarray(Wo)[:, sl].T.astype(np.float32)  # [192, 384]
        t = np.zeros((HPC, 128, C), np.float32)
        for h in range(HPC):
            blk = woT[HD * h:HD * (h + 1), :]
            t[h, 0:48] = blk
            t[h, 64:112] = blk
        return t.astype(BF16)

    return {
        "x1": x1f[b].astype(BF16),
        "x2": x2f[b].astype(BF16),
        "wqT": (np.asarray(Wq)[sl, :].T * SCALE).astype(BF16),
        "wkT": np.asarray(Wk)[sl, :].T.astype(BF16),
        "wv1T": np.asarray(Wv1)[sl, :].T.astype(BF16),
        "wv2T": np.asarray(Wv2)[sl, :].T.astype(BF16),
        "wo1T": wo_dup(Wo1),
        "wo2T": wo_dup(Wo2),
    }
